# revision 7
# baseline (speedup 1.0000x reference)
"""Trainium2 Bass kernel for ClebschCombiningSingleUnrolled (segment_reduce).

out[m, n, f] = sum_{m1+m2=m, m<7} cg[m1, m2] * X1[m1, n, f] * X2[m2, n, f]

Sharding: data-parallel along N (dim 1) across 8 NeuronCores; clebsch is
baked into the kernel as scalar immediates (compiled per cg value).
"""

import sys

if "/opt/trn_rl_repo" not in sys.path:
    sys.path.insert(0, "/opt/trn_rl_repo")

import numpy as np

import concourse.bass as bass
import concourse.bacc as bacc
import concourse.mybir as mybir
from concourse.tile import TileContext
from concourse.bass_utils import run_bass_kernel_spmd

# Problem constants (hardcoded per contest contract)
M = 7          # 2*lambd + 1 with lambd = 3
N = 2048
F = 2048
NCORES = 8
NS = N // NCORES           # N rows per core = 256
PART = 128                 # SBUF partitions
FD = 1024                  # free-dim elements per tile
ELEMS = NS * F             # elements per (m) plane per core = 524288
T = ELEMS // (PART * FD)   # tile iterations per core = 4

_VALID_PAIRS = [(m1, m - m1) for m in range(M) for m1 in range(m + 1)]


def build_nc(cg: np.ndarray, fd: int = FD) -> bass.Bass:
    """Build the per-core Bass module. cg values are baked as immediates."""
    f32 = mybir.dt.float32
    mult = mybir.AluOpType.mult

    # Bacc (not plain Bass): its generate_event_semaphores pass splits
    # multi-semaphore waits, which TRN2 compute instructions can't carry.
    nc = bacc.Bacc(None)
    x1 = nc.dram_tensor("X1", [M, NS, F], f32, kind="ExternalInput")
    x2 = nc.dram_tensor("X2", [M, NS, F], f32, kind="ExternalInput")
    out = nc.dram_tensor("out", [M, NS, F], f32, kind="ExternalOutput")

    t_iters = ELEMS // (PART * fd)
    # [M, T, 128, fd] views; per-partition lines are fd*4 contiguous bytes
    x1v = x1[:].rearrange("m n f -> m (n f)").rearrange(
        "m (t p c) -> m t p c", p=PART, c=fd
    )
    x2v = x2[:].rearrange("m n f -> m (n f)").rearrange(
        "m (t p c) -> m t p c", p=PART, c=fd
    )
    outv = out[:].rearrange("m n f -> m (n f)").rearrange(
        "m (t p c) -> m t p c", p=PART, c=fd
    )

    add = mybir.AluOpType.add

    with TileContext(nc) as tc:
        with (
            tc.tile_pool(name="ins", bufs=2) as pool_in,
            tc.tile_pool(name="accs", bufs=1) as pool_acc,
            tc.tile_pool(name="tmps", bufs=9) as pool_tmp,
        ):
            for t in range(t_iters):
                x1_t = []
                x2_t = []
                for m in range(M):
                    a = pool_in.tile([PART, fd], f32, tag=f"x1_{m}")
                    nc.sync.dma_start(out=a[:], in_=x1v[m, t])
                    x1_t.append(a)
                    b = pool_in.tile([PART, fd], f32, tag=f"x2_{m}")
                    nc.sync.dma_start(out=b[:], in_=x2v[m, t])
                    x2_t.append(b)

                # m = M-1 group first: its pairs (k, M-1-k) form a perfect
                # matching over all 14 input tiles, so these plain TT muls
                # are the ops that absorb every DMA-load semaphore wait.
                # (The STT ISA struct only has room for a single sync wait,
                # so STT instructions below must never carry cross-engine
                # deps: they read DVE-produced tiles only.)
                mtop = M - 1
                tops = []
                for m1 in range(mtop + 1):
                    p = pool_tmp.tile([PART, fd], f32, tag="tmp")
                    nc.vector.tensor_mul(
                        out=p[:], in0=x1_t[m1][:], in1=x2_t[mtop - m1][:]
                    )
                    tops.append(p)
                acc6 = pool_acc.tile([PART, fd], f32, tag=f"acc_{mtop}")
                # tensor_scalar carries the acc-slot WAR wait
                nc.vector.tensor_scalar_mul(
                    acc6[:], tops[0][:], float(cg[0, mtop])
                )
                for m1 in range(1, mtop + 1):
                    nc.vector.scalar_tensor_tensor(
                        acc6[:], tops[m1][:], float(cg[m1, mtop - m1]),
                        acc6[:], mult, add,
                    )
                nc.sync.dma_start(out=outv[mtop, t], in_=acc6[:])

                for m in range(mtop):
                    terms = []
                    for m1 in range(m + 1):
                        m2 = m - m1
                        tmp = pool_tmp.tile([PART, fd], f32, tag="tmp")
                        nc.vector.scalar_tensor_tensor(
                            tmp[:], x1_t[m1][:], float(cg[m1, m2]),
                            x2_t[m2][:], mult, mult,
                        )
                        terms.append(tmp)
                    if m == 0:
                        nc.sync.dma_start(out=outv[m, t], in_=terms[0][:])
                        continue
                    acc = pool_acc.tile([PART, fd], f32, tag=f"acc_{m}")
                    # first writer of the acc slot is a TT add (WAR wait ok)
                    nc.vector.tensor_add(
                        out=acc[:], in0=terms[0][:], in1=terms[1][:]
                    )
                    for k in range(2, m + 1):
                        nc.vector.tensor_add(
                            out=acc[:], in0=acc[:], in1=terms[k][:]
                        )
                    nc.sync.dma_start(out=outv[m, t], in_=acc[:])
    nc.finalize()  # Bacc.finalize runs compile(): wait-splitting, reg alloc
    return nc


def _shard_inputs(X1: np.ndarray, X2: np.ndarray) -> list[dict]:
    in_maps = []
    for i in range(NCORES):
        sl = slice(i * NS, (i + 1) * NS)
        in_maps.append(
            {
                "X1": np.ascontiguousarray(X1[:, sl, :], dtype=np.float32),
                "X2": np.ascontiguousarray(X2[:, sl, :], dtype=np.float32),
            }
        )
    return in_maps


def run(X1, X2, clebsch, trace: bool = False, **trace_kwargs):
    """Build, compile and run on 8 cores. Returns (output, BassKernelResults)."""
    X1 = np.asarray(X1, dtype=np.float32)
    X2 = np.asarray(X2, dtype=np.float32)
    cg = np.asarray(clebsch, dtype=np.float32)
    assert X1.shape == (M, N, F) and X2.shape == (M, N, F)
    assert cg.shape == (M, M)

    nc = build_nc(cg)
    in_maps = _shard_inputs(X1, X2)
    res = run_bass_kernel_spmd(
        nc, in_maps, core_ids=list(range(NCORES)), trace=trace, **trace_kwargs
    )
    shards = [np.asarray(r["out"]).reshape(M, NS, F) for r in res.results]
    full = np.concatenate(shards, axis=1)
    return full, res


def kernel(X1, X2, clebsch, lambd=3, **_unused) -> np.ndarray:
    out, _ = run(X1, X2, clebsch)
    return out.astype(np.float32)


# revision 10
# speedup vs baseline: 1.2649x; 1.2649x over previous
"""Trainium2 Bass kernel for ClebschCombiningSingleUnrolled (segment_reduce).

out[m, n, f] = sum_{m1+m2=m, m<7} cg[m1, m2] * X1[m1, n, f] * X2[m2, n, f]

Sharding: data-parallel along N (dim 1) across 8 NeuronCores; clebsch is
baked into the kernel as scalar immediates (compiled per cg value).
"""

import sys

if "/opt/trn_rl_repo" not in sys.path:
    sys.path.insert(0, "/opt/trn_rl_repo")

import numpy as np

import concourse.bass as bass
import concourse.bacc as bacc
import concourse.mybir as mybir
from concourse.tile import TileContext
from concourse.bass_utils import run_bass_kernel_spmd

# Problem constants (hardcoded per contest contract)
M = 7          # 2*lambd + 1 with lambd = 3
N = 2048
F = 2048
NCORES = 8
NS = N // NCORES           # N rows per core = 256
PART = 128                 # SBUF partitions
FD = 1024                  # free-dim elements per tile
ELEMS = NS * F             # elements per (m) plane per core = 524288
T = ELEMS // (PART * FD)   # tile iterations per core = 4

_VALID_PAIRS = [(m1, m - m1) for m in range(M) for m1 in range(m + 1)]


def build_nc(cg: np.ndarray, fd: int = FD) -> bass.Bass:
    """Build the per-core Bass module. cg values are baked as immediates."""
    f32 = mybir.dt.float32
    mult = mybir.AluOpType.mult

    # Bacc (not plain Bass): its generate_event_semaphores pass splits
    # multi-semaphore waits, which TRN2 compute instructions can't carry.
    nc = bacc.Bacc(None)
    x1 = nc.dram_tensor("X1", [M, NS, F], f32, kind="ExternalInput")
    x2 = nc.dram_tensor("X2", [M, NS, F], f32, kind="ExternalInput")
    out = nc.dram_tensor("out", [M, NS, F], f32, kind="ExternalOutput")

    t_iters = ELEMS // (PART * fd)
    # [M, T, 128, fd] views; per-partition lines are fd*4 contiguous bytes
    x1v = x1[:].rearrange("m n f -> m (n f)").rearrange(
        "m (t p c) -> m t p c", p=PART, c=fd
    )
    x2v = x2[:].rearrange("m n f -> m (n f)").rearrange(
        "m (t p c) -> m t p c", p=PART, c=fd
    )
    outv = out[:].rearrange("m n f -> m (n f)").rearrange(
        "m (t p c) -> m t p c", p=PART, c=fd
    )

    add = mybir.AluOpType.add

    with TileContext(nc) as tc:
        with (
            tc.tile_pool(name="ins", bufs=2) as pool_in,
            tc.tile_pool(name="accs", bufs=1) as pool_acc,
            tc.tile_pool(name="tmps", bufs=9) as pool_tmp,
        ):
            for t in range(t_iters):
                x1_t = []
                x2_t = []
                for m in range(M):
                    a = pool_in.tile([PART, fd], f32, tag=f"x1_{m}")
                    nc.sync.dma_start(out=a[:], in_=x1v[m, t])
                    x1_t.append(a)
                    b = pool_in.tile([PART, fd], f32, tag=f"x2_{m}")
                    nc.sync.dma_start(out=b[:], in_=x2v[m, t])
                    x2_t.append(b)

                # m = M-1 group first: its pairs (k, M-1-k) form a perfect
                # matching over all 14 input tiles, so these plain TT muls
                # are the ops that absorb every DMA-load semaphore wait.
                # (The STT ISA struct only has room for a single sync wait,
                # so STT instructions below must never carry cross-engine
                # deps: they read DVE-produced tiles only.)
                mtop = M - 1
                tops = []
                for m1 in range(mtop + 1):
                    p = pool_tmp.tile([PART, fd], f32, tag="tmp")
                    nc.vector.tensor_mul(
                        out=p[:], in0=x1_t[m1][:], in1=x2_t[mtop - m1][:]
                    )
                    tops.append(p)
                acc6 = pool_acc.tile([PART, fd], f32, tag=f"acc_{mtop}")
                # tensor_scalar carries the acc-slot WAR wait
                nc.vector.tensor_scalar_mul(
                    acc6[:], tops[0][:], float(cg[0, mtop])
                )
                for m1 in range(1, mtop + 1):
                    nc.vector.scalar_tensor_tensor(
                        acc6[:], tops[m1][:], float(cg[m1, mtop - m1]),
                        acc6[:], mult, add,
                    )
                nc.sync.dma_start(out=outv[mtop, t], in_=acc6[:])

                for m in range(mtop):
                    terms = []
                    for m1 in range(m + 1):
                        m2 = m - m1
                        tmp = pool_tmp.tile([PART, fd], f32, tag="tmp")
                        nc.vector.scalar_tensor_tensor(
                            tmp[:], x1_t[m1][:], float(cg[m1, m2]),
                            x2_t[m2][:], mult, mult,
                        )
                        terms.append(tmp)
                    if m == 0:
                        nc.sync.dma_start(out=outv[m, t], in_=terms[0][:])
                        continue
                    acc = pool_acc.tile([PART, fd], f32, tag=f"acc_{m}")
                    # first writer of the acc slot is a TT add (WAR wait ok)
                    nc.vector.tensor_add(
                        out=acc[:], in0=terms[0][:], in1=terms[1][:]
                    )
                    for k in range(2, m + 1):
                        nc.vector.tensor_add(
                            out=acc[:], in0=acc[:], in1=terms[k][:]
                        )
                    nc.sync.dma_start(out=outv[m, t], in_=acc[:])
    nc.finalize()  # Bacc.finalize runs compile(): wait-splitting, reg alloc
    return nc


def build_nc_f16(cg: np.ndarray, fd: int = FD) -> bass.Bass:
    """fp16 compute path: ACT casts fp32<->fp16; DVE runs 2x-mode fp16 ops.
    28 fused (X1*cg)*X2 products + 21 tree adds per tile iteration."""
    f32 = mybir.dt.float32
    f16 = mybir.dt.float16
    mult = mybir.AluOpType.mult

    nc = bacc.Bacc(None)
    x1 = nc.dram_tensor("X1", [M, NS, F], f32, kind="ExternalInput")
    x2 = nc.dram_tensor("X2", [M, NS, F], f32, kind="ExternalInput")
    out = nc.dram_tensor("out", [M, NS, F], f32, kind="ExternalOutput")

    t_iters = ELEMS // (PART * fd)
    x1v = x1[:].rearrange("m n f -> m (n f)").rearrange(
        "m (t p c) -> m t p c", p=PART, c=fd
    )
    x2v = x2[:].rearrange("m n f -> m (n f)").rearrange(
        "m (t p c) -> m t p c", p=PART, c=fd
    )
    outv = out[:].rearrange("m n f -> m (n f)").rearrange(
        "m (t p c) -> m t p c", p=PART, c=fd
    )

    with TileContext(nc) as tc:
        with (
            tc.tile_pool(name="stage", bufs=6) as pool_st,
            tc.tile_pool(name="ins16", bufs=2) as pool_in,
            tc.tile_pool(name="tmp16", bufs=10) as pool_tmp,
            tc.tile_pool(name="ost", bufs=4) as pool_ost,
        ):
            for t in range(t_iters):
                x1h = []
                x2h = []
                for m in range(M):
                    s = pool_st.tile([PART, fd], f32, tag="st")
                    nc.sync.dma_start(out=s[:], in_=x1v[m, t])
                    h = pool_in.tile([PART, fd], f16, tag=f"x1_{m}")
                    nc.scalar.copy(out=h[:], in_=s[:])
                    x1h.append(h)
                    s = pool_st.tile([PART, fd], f32, tag="st")
                    nc.sync.dma_start(out=s[:], in_=x2v[m, t])
                    h = pool_in.tile([PART, fd], f16, tag=f"x2_{m}")
                    nc.scalar.copy(out=h[:], in_=s[:])
                    x2h.append(h)
                for m in range(M):
                    terms = []
                    for m1 in range(m + 1):
                        m2 = m - m1
                        tmp = pool_tmp.tile([PART, fd], f16, tag="tmp")
                        nc.vector.scalar_tensor_tensor(
                            tmp[:], x1h[m1][:], float(cg[m1, m2]),
                            x2h[m2][:], mult, mult,
                        )
                        terms.append(tmp)
                    # pairwise tree reduction (fp16 2x adds)
                    while len(terms) > 1:
                        nxt = []
                        for k in range(0, len(terms) - 1, 2):
                            s2 = pool_tmp.tile([PART, fd], f16, tag="tmp")
                            nc.vector.tensor_add(
                                out=s2[:], in0=terms[k][:], in1=terms[k + 1][:]
                            )
                            nxt.append(s2)
                        if len(terms) % 2:
                            nxt.append(terms[-1])
                        terms = nxt
                    o = pool_ost.tile([PART, fd], f32, tag="ost")
                    nc.scalar.copy(out=o[:], in_=terms[0][:])
                    nc.sync.dma_start(out=outv[m, t], in_=o[:])
    nc.finalize()
    return nc


def _shard_inputs(X1: np.ndarray, X2: np.ndarray) -> list[dict]:
    in_maps = []
    for i in range(NCORES):
        sl = slice(i * NS, (i + 1) * NS)
        in_maps.append(
            {
                "X1": np.ascontiguousarray(X1[:, sl, :], dtype=np.float32),
                "X2": np.ascontiguousarray(X2[:, sl, :], dtype=np.float32),
            }
        )
    return in_maps


VARIANT = "f16"  # "f32" | "f16"


def run(X1, X2, clebsch, trace: bool = False, variant: str | None = None,
        **trace_kwargs):
    """Build, compile and run on 8 cores. Returns (output, BassKernelResults)."""
    X1 = np.asarray(X1, dtype=np.float32)
    X2 = np.asarray(X2, dtype=np.float32)
    cg = np.asarray(clebsch, dtype=np.float32)
    assert X1.shape == (M, N, F) and X2.shape == (M, N, F)
    assert cg.shape == (M, M)

    variant = variant or VARIANT
    nc = build_nc(cg) if variant == "f32" else build_nc_f16(cg)
    in_maps = _shard_inputs(X1, X2)
    res = run_bass_kernel_spmd(
        nc, in_maps, core_ids=list(range(NCORES)), trace=trace, **trace_kwargs
    )
    shards = [np.asarray(r["out"]).reshape(M, NS, F) for r in res.results]
    full = np.concatenate(shards, axis=1)
    return full, res


def kernel(X1, X2, clebsch, lambd=3, **_unused) -> np.ndarray:
    out, _ = run(X1, X2, clebsch)
    return out.astype(np.float32)


# revision 14
# speedup vs baseline: 1.5141x; 1.1971x over previous
"""Trainium2 Bass kernel for ClebschCombiningSingleUnrolled (segment_reduce).

out[m, n, f] = sum_{m1+m2=m, m<7} cg[m1, m2] * X1[m1, n, f] * X2[m2, n, f]

Sharding: data-parallel along N (dim 1) across 8 NeuronCores; clebsch is
baked into the kernel as scalar immediates (compiled per cg value).
"""

import sys

if "/opt/trn_rl_repo" not in sys.path:
    sys.path.insert(0, "/opt/trn_rl_repo")

import numpy as np

import concourse.bass as bass
import concourse.bacc as bacc
import concourse.mybir as mybir
from concourse.tile import TileContext
from concourse.bass_utils import run_bass_kernel_spmd

# Problem constants (hardcoded per contest contract)
M = 7          # 2*lambd + 1 with lambd = 3
N = 2048
F = 2048
NCORES = 8
NS = N // NCORES           # N rows per core = 256
PART = 128                 # SBUF partitions
FD = 1024                  # free-dim elements per tile
ELEMS = NS * F             # elements per (m) plane per core = 524288
T = ELEMS // (PART * FD)   # tile iterations per core = 4

_VALID_PAIRS = [(m1, m - m1) for m in range(M) for m1 in range(m + 1)]


def build_nc(cg: np.ndarray, fd: int = FD) -> bass.Bass:
    """Build the per-core Bass module. cg values are baked as immediates."""
    f32 = mybir.dt.float32
    mult = mybir.AluOpType.mult

    # Bacc (not plain Bass): its generate_event_semaphores pass splits
    # multi-semaphore waits, which TRN2 compute instructions can't carry.
    nc = bacc.Bacc(None)
    x1 = nc.dram_tensor("X1", [M, NS, F], f32, kind="ExternalInput")
    x2 = nc.dram_tensor("X2", [M, NS, F], f32, kind="ExternalInput")
    out = nc.dram_tensor("out", [M, NS, F], f32, kind="ExternalOutput")

    t_iters = ELEMS // (PART * fd)
    # [M, T, 128, fd] views; per-partition lines are fd*4 contiguous bytes
    x1v = x1[:].rearrange("m n f -> m (n f)").rearrange(
        "m (t p c) -> m t p c", p=PART, c=fd
    )
    x2v = x2[:].rearrange("m n f -> m (n f)").rearrange(
        "m (t p c) -> m t p c", p=PART, c=fd
    )
    outv = out[:].rearrange("m n f -> m (n f)").rearrange(
        "m (t p c) -> m t p c", p=PART, c=fd
    )

    add = mybir.AluOpType.add

    with TileContext(nc) as tc:
        with (
            tc.tile_pool(name="ins", bufs=2) as pool_in,
            tc.tile_pool(name="accs", bufs=1) as pool_acc,
            tc.tile_pool(name="tmps", bufs=9) as pool_tmp,
        ):
            for t in range(t_iters):
                x1_t = []
                x2_t = []
                for m in range(M):
                    a = pool_in.tile([PART, fd], f32, tag=f"x1_{m}")
                    nc.sync.dma_start(out=a[:], in_=x1v[m, t])
                    x1_t.append(a)
                    b = pool_in.tile([PART, fd], f32, tag=f"x2_{m}")
                    nc.sync.dma_start(out=b[:], in_=x2v[m, t])
                    x2_t.append(b)

                # m = M-1 group first: its pairs (k, M-1-k) form a perfect
                # matching over all 14 input tiles, so these plain TT muls
                # are the ops that absorb every DMA-load semaphore wait.
                # (The STT ISA struct only has room for a single sync wait,
                # so STT instructions below must never carry cross-engine
                # deps: they read DVE-produced tiles only.)
                mtop = M - 1
                tops = []
                for m1 in range(mtop + 1):
                    p = pool_tmp.tile([PART, fd], f32, tag="tmp")
                    nc.vector.tensor_mul(
                        out=p[:], in0=x1_t[m1][:], in1=x2_t[mtop - m1][:]
                    )
                    tops.append(p)
                acc6 = pool_acc.tile([PART, fd], f32, tag=f"acc_{mtop}")
                # tensor_scalar carries the acc-slot WAR wait
                nc.vector.tensor_scalar_mul(
                    acc6[:], tops[0][:], float(cg[0, mtop])
                )
                for m1 in range(1, mtop + 1):
                    nc.vector.scalar_tensor_tensor(
                        acc6[:], tops[m1][:], float(cg[m1, mtop - m1]),
                        acc6[:], mult, add,
                    )
                nc.sync.dma_start(out=outv[mtop, t], in_=acc6[:])

                for m in range(mtop):
                    terms = []
                    for m1 in range(m + 1):
                        m2 = m - m1
                        tmp = pool_tmp.tile([PART, fd], f32, tag="tmp")
                        nc.vector.scalar_tensor_tensor(
                            tmp[:], x1_t[m1][:], float(cg[m1, m2]),
                            x2_t[m2][:], mult, mult,
                        )
                        terms.append(tmp)
                    if m == 0:
                        nc.sync.dma_start(out=outv[m, t], in_=terms[0][:])
                        continue
                    acc = pool_acc.tile([PART, fd], f32, tag=f"acc_{m}")
                    # first writer of the acc slot is a TT add (WAR wait ok)
                    nc.vector.tensor_add(
                        out=acc[:], in0=terms[0][:], in1=terms[1][:]
                    )
                    for k in range(2, m + 1):
                        nc.vector.tensor_add(
                            out=acc[:], in0=acc[:], in1=terms[k][:]
                        )
                    nc.sync.dma_start(out=outv[m, t], in_=acc[:])
    nc.finalize()  # Bacc.finalize runs compile(): wait-splitting, reg alloc
    return nc


def build_nc_f16(cg: np.ndarray, fd: int = FD, act_scale_min_m1: int = 2) -> bass.Bass:
    """fp16 compute path.

    STT has no 2x uop on cayman (measured 1216ns vs TT's 685ns), so products
    are plain TT muls at 2x and the cg scale is pre-applied to the X1 operand:
      - pairs with m1 >= act_scale_min_m1: ACT makes a scaled fp32->fp16 cast
        per pair (activation Copy with scale=cg), replacing those planes'
        base casts entirely.
      - pairs with m1 < act_scale_min_m1: DVE tensor_scalar (fp16 4x mode)
        from the base fp16 cast.
    Tree adds run fp16 at 2x; out-cast fp16->fp32 on ACT."""
    f32 = mybir.dt.float32
    f16 = mybir.dt.float16
    mult = mybir.AluOpType.mult

    nc = bacc.Bacc(None)
    x1 = nc.dram_tensor("X1", [M, NS, F], f32, kind="ExternalInput")
    x2 = nc.dram_tensor("X2", [M, NS, F], f32, kind="ExternalInput")
    out = nc.dram_tensor("out", [M, NS, F], f32, kind="ExternalOutput")

    t_iters = ELEMS // (PART * fd)
    x1v = x1[:].rearrange("m n f -> m (n f)").rearrange(
        "m (t p c) -> m t p c", p=PART, c=fd
    )
    x2v = x2[:].rearrange("m n f -> m (n f)").rearrange(
        "m (t p c) -> m t p c", p=PART, c=fd
    )
    outv = out[:].rearrange("m n f -> m (n f)").rearrange(
        "m (t p c) -> m t p c", p=PART, c=fd
    )

    with TileContext(nc) as tc:
        with (
            tc.tile_pool(name="stage", bufs=2) as pool_st,
            tc.tile_pool(name="ins16", bufs=2) as pool_in,
            tc.tile_pool(name="tmp16", bufs=10) as pool_tmp,
            tc.tile_pool(name="ost", bufs=4) as pool_ost,
        ):
            for t in range(t_iters):
                x1h = []        # base fp16 casts of X1 (only m1 < act_scale_min_m1)
                x1stage = []    # fp32 staging tiles for X1 (for ACT scaled casts)
                x2h = []
                for m in range(M):
                    s = pool_st.tile([PART, fd], f32, tag=f"st1_{m}")
                    nc.sync.dma_start(out=s[:], in_=x1v[m, t])
                    x1stage.append(s)
                    if m < act_scale_min_m1:
                        h = pool_in.tile([PART, fd], f16, tag=f"x1_{m}")
                        nc.scalar.copy(out=h[:], in_=s[:])
                        x1h.append(h)
                    else:
                        x1h.append(None)
                    s2 = pool_st.tile([PART, fd], f32, tag="st2", bufs=5)
                    nc.sync.dma_start(out=s2[:], in_=x2v[m, t])
                    h = pool_in.tile([PART, fd], f16, tag=f"x2_{m}")
                    nc.scalar.copy(out=h[:], in_=s2[:])
                    x2h.append(h)
                for m in range(M):
                    terms = []
                    for m1 in range(m + 1):
                        m2 = m - m1
                        c = float(cg[m1, m2])
                        ysc = pool_tmp.tile([PART, fd], f16, tag="ysc")
                        if m1 >= act_scale_min_m1:
                            # ACT: scaled cast straight from fp32 stage
                            nc.scalar.mul(ysc[:], x1stage[m1][:], c)
                        else:
                            # DVE: fp16 tensor_scalar at 4x
                            nc.vector.tensor_scalar_mul(ysc[:], x1h[m1][:], c)
                        tmp = pool_tmp.tile([PART, fd], f16, tag="tmp")
                        nc.vector.tensor_mul(
                            out=tmp[:], in0=ysc[:], in1=x2h[m2][:]
                        )
                        terms.append(tmp)
                    # pairwise tree reduction (fp16 2x adds)
                    while len(terms) > 1:
                        nxt = []
                        for k in range(0, len(terms) - 1, 2):
                            s2 = pool_tmp.tile([PART, fd], f16, tag="tmp")
                            nc.vector.tensor_add(
                                out=s2[:], in0=terms[k][:], in1=terms[k + 1][:]
                            )
                            nxt.append(s2)
                        if len(terms) % 2:
                            nxt.append(terms[-1])
                        terms = nxt
                    o = pool_ost.tile([PART, fd], f32, tag="ost")
                    nc.scalar.copy(out=o[:], in_=terms[0][:])
                    nc.sync.dma_start(out=outv[m, t], in_=o[:])
    nc.finalize()
    return nc


def _shard_inputs(X1: np.ndarray, X2: np.ndarray) -> list[dict]:
    in_maps = []
    for i in range(NCORES):
        sl = slice(i * NS, (i + 1) * NS)
        in_maps.append(
            {
                "X1": np.ascontiguousarray(X1[:, sl, :], dtype=np.float32),
                "X2": np.ascontiguousarray(X2[:, sl, :], dtype=np.float32),
            }
        )
    return in_maps


VARIANT = "f16"  # "f32" | "f16"


def run(X1, X2, clebsch, trace: bool = False, variant: str | None = None,
        **trace_kwargs):
    """Build, compile and run on 8 cores. Returns (output, BassKernelResults)."""
    X1 = np.asarray(X1, dtype=np.float32)
    X2 = np.asarray(X2, dtype=np.float32)
    cg = np.asarray(clebsch, dtype=np.float32)
    assert X1.shape == (M, N, F) and X2.shape == (M, N, F)
    assert cg.shape == (M, M)

    variant = variant or VARIANT
    nc = build_nc(cg) if variant == "f32" else build_nc_f16(cg)
    in_maps = _shard_inputs(X1, X2)
    res = run_bass_kernel_spmd(
        nc, in_maps, core_ids=list(range(NCORES)), trace=trace, **trace_kwargs
    )
    shards = [np.asarray(r["out"]).reshape(M, NS, F) for r in res.results]
    full = np.concatenate(shards, axis=1)
    return full, res


def kernel(X1, X2, clebsch, lambd=3, **_unused) -> np.ndarray:
    out, _ = run(X1, X2, clebsch)
    return out.astype(np.float32)
